# revision 20
# baseline (speedup 1.0000x reference)
"""Block-sparse linear y = x @ W^T + b on 8 TRN2 NeuronCores.

Problem shape (hardcoded): x [8192, 4096] f32, weight [1024, 64, 64] f32
(64x64 blocks), bias [4096] f32, row_idx/col_idx [1024] int32 over a 64x64
block grid.

Strategy: data-parallel over tokens (1024/core); per core compute
y^T = W x^T + b with bf16 matmuls on the PE array's four 64x64 quadrants
(tile_position), which stream concurrently at the full-array rate
(measured 216ns per 4-matmul round).

x^T is SBUF-resident ONCE: col-blocks are 2-colored so every out-row has
exactly half its blocks with x in the top partition half (-> PE row-group
0) and half in the bottom (-> row-group 1); overflow columns are
duplicated at the opposite half until every row splits evenly. Out-rows
are paired (2p, 2p+1) sharing one PSUM bank per token-half th (r1 ->
array col-group 0, r2 -> col-group 1). Pair couples (A, B) run in two
slots: slot s streams A's top-blocks on row-group 0 while B's
bottom-blocks stream on row-group 1; slot s+1 swaps. Banks accumulate
across both stages; 8 banks = 2 couples in flight + 1 evicting. x lives
in [128, 4*ntok] tiles (4 top + 4 bottom col-blocks) grouped by first
use, and each stage's block order follows tile rank, so compute starts
after ~0.5MB of x has landed. Eviction is one op (psum + bias -> bf16
SBUF) alternating scalar/vector, th0 banks mid-slot; all DMA triggers
ride HWDGE queues (sync: x/weights, scalar: x/outputs) - gpsimd is
unused, avoiding its 6.4us SWDGE end-of-kernel drain. (A PE p-state
warmup spin was tried and removed: its matmuls started late on a
semaphore and ran at half clock, delaying real work ~10us.)
"""

from contextlib import ExitStack

import numpy as np
import ml_dtypes

import concourse.tile as tile
from concourse import bacc, mybir
from concourse.bass_utils import run_bass_kernel_spmd

BLK = 64
OUT_BLK = 64
IN_BLK = 64
D_IN = IN_BLK * BLK    # 4096
D_OUT = OUT_BLK * BLK  # 4096
N_CORES = 8
XPACK = 4              # col-block pairs per x SBUF tile (split by th)
BF16 = ml_dtypes.bfloat16


def _dedupe(row_idx, col_idx):
    d = {}
    for i in range(len(row_idx)):
        d[(int(row_idx[i]), int(col_idx[i]))] = i
    blocks_by_r = [[] for _ in range(OUT_BLK)]
    for (r, c), w in d.items():
        blocks_by_r[r].append((c, w))
    for lst in blocks_by_r:
        lst.sort()
    return blocks_by_r


def _balance_color(blocks_by_r, seed=0):
    """2-color the 64 col-blocks minimizing sum |#top-blocks(r) - n_r/2|."""
    Mi = np.zeros((OUT_BLK, IN_BLK), np.int64)
    for r, lst in enumerate(blocks_by_r):
        for c, _ in lst:
            Mi[r, c] = 1
    tgt = np.array([len(l) / 2.0 for l in blocks_by_r])
    best = None
    for s in range(8):
        rs = np.random.default_rng(seed + s)
        color = (rs.random(IN_BLK) < 0.5).astype(np.int8)  # 1 = top
        e = Mi[:, color == 1].sum(1).astype(float)
        c = float(np.abs(e - tgt).sum())
        T = 2.0
        for _ in range(40000):
            if c < 1e-9:
                break
            i = int(rs.integers(IN_BLK))
            ne = e + Mi[:, i] * (1 - 2 * color[i])
            ncst = float(np.abs(ne - tgt).sum())
            if ncst <= c or rs.random() < np.exp((c - ncst) / max(T, 1e-9)):
                color[i] ^= 1
                e, c = ne, ncst
            T *= 0.9997
        if best is None or c < best[0]:
            best = (c, color.copy())
        if c < 1e-9:
            break
    return best[1]


def _assign_stages(blocks_by_r, color):
    """Per-block stage (0=top/ki0, 1=bottom/ki1); duplicate overflow cols
    at the opposite half until every row splits ceil/floor(n/2)."""
    stage_of = {}
    for r, lst in enumerate(blocks_by_r):
        for c, _ in lst:
            stage_of[(r, c)] = 0 if color[c] == 1 else 1
    dup_top = set()   # cols (colored bottom) also available at a top half
    dup_bot = set()
    for _ in range(64):
        moved = False
        devs = []
        for r, lst in enumerate(blocks_by_r):
            n = len(lst)
            k0 = sum(1 for c, _ in lst if stage_of[(r, c)] == 0)
            devs.append(k0 - (n + 1) // 2 if k0 > n // 2 else k0 - n // 2
                        if k0 < n // 2 else 0)
        # free moves via existing dups
        for r, lst in enumerate(blocks_by_r):
            d = devs[r]
            while d > 0:
                c = next((c for c, _ in lst if stage_of[(r, c)] == 0
                          and c in dup_bot), None)
                if c is None:
                    break
                stage_of[(r, c)] = 1
                d -= 1
                moved = True
            while d < 0:
                c = next((c for c, _ in lst if stage_of[(r, c)] == 1
                          and c in dup_top), None)
                if c is None:
                    break
                stage_of[(r, c)] = 0
                d += 1
                moved = True
            devs[r] = d
        if all(d == 0 for d in devs):
            break
        if not moved:
            # add the dup col helping the most deficient rows
            cnt_b, cnt_t = {}, {}
            for r, lst in enumerate(blocks_by_r):
                if devs[r] > 0:
                    for c, _ in lst:
                        if stage_of[(r, c)] == 0 and c not in dup_bot:
                            cnt_b[c] = cnt_b.get(c, 0) + 1
                elif devs[r] < 0:
                    for c, _ in lst:
                        if stage_of[(r, c)] == 1 and c not in dup_top:
                            cnt_t[c] = cnt_t.get(c, 0) + 1
            if cnt_b and (not cnt_t or max(cnt_b.values())
                          >= max(cnt_t.values())):
                dup_bot.add(max(cnt_b, key=cnt_b.get))
            elif cnt_t:
                dup_top.add(max(cnt_t, key=cnt_t.get))
            else:
                break
    stage_lists = []
    for r, lst in enumerate(blocks_by_r):
        l0 = [(c, w) for c, w in lst if stage_of[(r, c)] == 0]
        l1 = [(c, w) for c, w in lst if stage_of[(r, c)] == 1]
        stage_lists.append((l0, l1))
    return stage_lists, sorted(dup_top), sorted(dup_bot)


def _couple_order(stage_lists, color):
    """Order pairs into couples; greedy to reuse col-blocks early."""
    npairs = OUT_BLK // 2
    cols_of = []
    for p in range(npairs):
        s = set()
        for r in (2 * p, 2 * p + 1):
            for ki in (0, 1):
                for c, _ in stage_lists[r][ki]:
                    s.add((c, ki))
        cols_of.append(s)
    remaining = set(range(npairs))
    order = []
    seen = set()
    cur = 0
    while remaining:
        order.append(cur)
        remaining.discard(cur)
        seen |= cols_of[cur]
        if not remaining:
            break
        cur = min(remaining, key=lambda p: len(cols_of[p] - seen))
    return order


def _layout_tiles(stage_lists, color, dup_top, dup_bot, porder):
    """Group (top-col, bottom-col) into XPACK-wide tiles by first-use slot;
    reorder each stage list by tile rank. Returns tiles, loc, stage_lists."""
    nslots = len(porder)
    first_top, first_bot = {}, {}
    for s in range(nslots):
        A, B = porder[s], porder[s ^ 1]
        for mi in (0, 1):
            for c, _ in stage_lists[2 * A + mi][0]:
                first_top.setdefault(c, (s, len(first_top)))
            for c, _ in stage_lists[2 * B + mi][1]:
                first_bot.setdefault(c, (s, len(first_bot)))
    top_cols = sorted(first_top, key=first_top.get)       # used at a top half
    bot_cols = sorted(first_bot, key=first_bot.get)
    nslots_x = max(len(top_cols), len(bot_cols))
    slots = []
    for i in range(nslots_x):
        tc = top_cols[i] if i < len(top_cols) else None
        bc = bot_cols[i] if i < len(bot_cols) else None
        slots.append((tc, bc))
    ntiles = (nslots_x + XPACK - 1) // XPACK
    tiles = [slots[t * XPACK:(t + 1) * XPACK] for t in range(ntiles)]
    loc = {}
    for t, sl in enumerate(tiles):
        for j, (tc, bc) in enumerate(sl):
            if tc is not None:
                loc[(tc, 0)] = (t, j)
            if bc is not None:
                loc[(bc, 1)] = (t, j)
    # tiles are th-split at DMA/SBUF level: tile (t, th) holds the th
    # token-half of XPACK col-pairs; loc is th-independent
    out_lists = []
    for r, (l0, l1) in enumerate(stage_lists):
        out_lists.append((sorted(l0, key=lambda cw: loc[(cw[0], 0)]),
                          sorted(l1, key=lambda cw: loc[(cw[0], 1)])))
    return tiles, loc, out_lists


def _pack_host_arrays(weight, bias, stage_lists, porder):
    nslots = len(porder)
    widths = []
    for s in range(nslots):
        A, B = porder[s], porder[s ^ 1]
        w = 1
        for mi in (0, 1):
            w = max(w, len(stage_lists[2 * A + mi][0]),
                    len(stage_lists[2 * B + mi][1]))
        widths.append(w * 2)
    offs = np.cumsum([0] + widths)
    wpk = np.zeros((128, int(offs[-1]) * BLK), dtype=BF16)
    wT = np.ascontiguousarray(
        np.transpose(np.asarray(weight), (0, 2, 1))).astype(BF16)
    for s in range(nslots):
        base = int(offs[s])
        A, B = porder[s], porder[s ^ 1]
        for ki, p in ((0, A), (1, B)):
            for mi in (0, 1):
                for b, (c, w) in enumerate(stage_lists[2 * p + mi][ki]):
                    col = (base + 2 * b + mi) * BLK
                    wpk[ki * 64:(ki + 1) * 64, col:col + BLK] = wT[w]
    bias_pk = np.zeros((128, OUT_BLK // 2), dtype=np.float32)
    for p in range(OUT_BLK // 2):
        bias_pk[0:64, p] = bias[2 * p * BLK:(2 * p + 1) * BLK]
        bias_pk[64:128, p] = bias[(2 * p + 1) * BLK:(2 * p + 2) * BLK]
    return wpk, bias_pk, offs


def _build_kernel(stage_lists, tiles, loc, porder, offs, ntok,
                  w_bufs=6, out_bufs=8):
    n_th = ntok // 512
    assert n_th == 2
    sdt = mybir.dt.bfloat16
    f32 = mybir.dt.float32
    nslots = len(porder)
    ntiles = len(tiles)

    nc = bacc.Bacc("TRN2", target_bir_lowering=False, debug=False)
    xt_d = nc.dram_tensor("xt", [ntiles * 2 * 128, XPACK * (ntok // 2)], sdt,
                          kind="ExternalInput").ap()
    w_d = nc.dram_tensor("wpk", [128, int(offs[-1]) * BLK], sdt,
                         kind="ExternalInput").ap()
    bias_d = nc.dram_tensor("bias_pk", [128, OUT_BLK // 2], f32,
                            kind="ExternalInput").ap()
    yt_d = nc.dram_tensor("yt", [D_OUT, ntok], sdt,
                          kind="ExternalOutput").ap()

    with tile.TileContext(nc) as tc:
        with ExitStack() as ctx:
            xpool = ctx.enter_context(tc.tile_pool(name="xp", bufs=1))
            wpool = ctx.enter_context(tc.tile_pool(name="wp", bufs=w_bufs))
            pspool = ctx.enter_context(
                tc.tile_pool(name="ps", bufs=8, space="PSUM"))
            opool = ctx.enter_context(tc.tile_pool(name="op", bufs=out_bufs))
            bpool = ctx.enter_context(tc.tile_pool(name="bp", bufs=1))

            bias_sb = bpool.tile([128, OUT_BLK // 2], f32, tag="bias",
                                 name="bias_sb")

            xtiles = {}
            nxdma = [0]

            def x_ap(c, ki, th):
                t, j = loc[(c, ki)]
                key = (t, th)
                if key not in xtiles:
                    tl = xpool.tile([128, XPACK * 512], sdt,
                                    tag=f"x{t}_{th}", name=f"x{t}_{th}")
                    eng = nc.sync if nxdma[0] % 2 == 0 else nc.scalar
                    nxdma[0] += 1
                    row = (t * 2 + th) * 128
                    eng.dma_start(tl[:], xt_d[row:row + 128, :])
                    xtiles[key] = tl
                tl = xtiles[key]
                o = j * 512
                return tl[ki * 64:(ki + 1) * 64, o:o + 512]

            def slot_cols(s):
                cols = []
                A, B = porder[s], porder[s ^ 1]
                for ki, p in ((0, A), (1, B)):
                    for mi in (0, 1):
                        for c, _w in stage_lists[2 * p + mi][ki]:
                            cols.append((c, ki))
                return cols

            s0_cols = slot_cols(0)
            s1_cols = slot_cols(1)

            wg_tiles = {}

            def ensure_wg(s):
                if s < nslots and s not in wg_tiles:
                    ncols = int(offs[s + 1] - offs[s]) * BLK
                    t = wpool.tile([128, ncols], sdt, tag="wg",
                                   name=f"wg{s}")
                    nc.sync.dma_start(
                        t[:], w_d[:, int(offs[s]) * BLK:
                                  int(offs[s]) * BLK + ncols])
                    wg_tiles[s] = t

            psum = {}
            mmidx = {}
            mmtot = {}
            for p in range(OUT_BLK // 2):
                for mi in (0, 1):
                    mmtot[(p, mi)] = (len(stage_lists[2 * p + mi][0])
                                      + len(stage_lists[2 * p + mi][1]))

            # Startup-critical DMA order: wg0, slot-0 th0 x, wg1, slot-1
            # th0 x, bias, then th1 tiles; the slot loop fetches the rest
            # in first-use order.
            ensure_wg(0)
            for c, ki in s0_cols:
                x_ap(c, ki, 0)
            ensure_wg(1)
            for c, ki in s1_cols:
                x_ap(c, ki, 0)
            nc.sync.dma_start(bias_sb[:], bias_d[:])
            for c, ki in s0_cols:
                x_ap(c, ki, 1)
            for c, ki in s1_cols:
                x_ap(c, ki, 1)

            nev = [0]

            def evict(p, th):
                pt = psum.pop((p, th))
                osb = opool.tile([128, 512], sdt, tag="o", name=f"o{p}_{th}")
                nev[0] += 1
                bcol = bias_sb[:, p:p + 1]
                n1 = mmtot[(p, 0)]
                n2 = mmtot[(p, 1)]
                if n1 > 0 and n2 > 0:
                    nc.vector.tensor_scalar_add(osb[:], pt[:], bcol)
                else:
                    for mi, nm in ((0, n1), (1, n2)):
                        oh = osb[mi * 64:(mi + 1) * 64, :]
                        bh = bias_sb[mi * 64:(mi + 1) * 64, p:p + 1]
                        if nm > 0:
                            nc.vector.tensor_scalar_add(
                                oh, pt[mi * 64:(mi + 1) * 64, :], bh)
                        else:
                            nc.vector.memset(oh, 0.0)
                            nc.vector.tensor_scalar_add(oh, oh, bh)
                nc.scalar.dma_start(
                    yt_d[2 * p * BLK:2 * p * BLK + 128,
                         th * 512:(th + 1) * 512],
                    osb[:])

            def ensure_psum(p, th):
                if (p, th) not in psum:
                    psum[(p, th)] = pspool.tile(
                        [128, 512], f32, tag="ps", name=f"ps{p}_{th}")
                    for mi in (0, 1):
                        mmidx[(p, th, mi)] = 0

            for s in range(nslots):
                ensure_wg(s)
                ensure_wg(s + 1)
                ensure_wg(s + 2)
                base = int(offs[s])
                wg = wg_tiles[s]
                A, B = porder[s], porder[s ^ 1]
                work = []
                for ki, p in ((0, A), (1, B)):
                    for mi in (0, 1):
                        work.append((ki, p, mi, stage_lists[2 * p + mi][ki]))
                    for th in range(n_th):
                        ensure_psum(p, th)
                nsteps = max((len(w[3]) for w in work), default=1)
                for th in range(n_th):
                    for b in range(nsteps):
                        for ki, p, mi, blks in work:
                            if b >= len(blks):
                                continue
                            c, _w = blks[b]
                            lhsT = wg[ki * 64:(ki + 1) * 64,
                                      (2 * b + mi) * BLK:
                                      (2 * b + mi + 1) * BLK]
                            i = mmidx[(p, th, mi)]
                            mmidx[(p, th, mi)] = i + 1
                            nc.tensor.matmul(
                                psum[(p, th)][mi * 64:(mi + 1) * 64, :],
                                lhsT, x_ap(c, ki, th),
                                start=(i == 0),
                                stop=(i == mmtot[(p, mi)] - 1),
                                tile_position=(ki * 64, mi * 64),
                                skip_group_check=True,
                            )
                    if s % 2 == 1:
                        # both couple pairs' th banks complete here
                        for p in (porder[s - 1], porder[s]):
                            evict(p, th)
    nc.compile()
    return nc


def kernel(x, weight, bias, row_idx, col_idx):
    x = np.asarray(x, dtype=np.float32)
    weight = np.asarray(weight, dtype=np.float32)
    bias = np.asarray(bias, dtype=np.float32)
    row_idx = np.asarray(row_idx)
    col_idx = np.asarray(col_idx)
    ntok_total = x.shape[0]
    assert ntok_total % N_CORES == 0
    ntok = ntok_total // N_CORES

    blocks_by_r = _dedupe(row_idx, col_idx)
    color = _balance_color(blocks_by_r)
    stage_lists, dup_top, dup_bot = _assign_stages(blocks_by_r, color)
    porder = _couple_order(stage_lists, color)
    tiles, loc, stage_lists = _layout_tiles(
        stage_lists, color, dup_top, dup_bot, porder)
    wpk, bias_pk, offs = _pack_host_arrays(
        weight, bias, stage_lists, porder)
    nc = _build_kernel(stage_lists, tiles, loc, porder, offs, ntok)

    in_maps = []
    for cid in range(N_CORES):
        xT = np.ascontiguousarray(
            x[cid * ntok:(cid + 1) * ntok].T).astype(BF16)  # [4096, ntok]
        half = ntok // 2
        xt = np.zeros((len(tiles) * 2 * 128, XPACK * half), dtype=BF16)
        for t, sl in enumerate(tiles):
            for th in range(2):
                r0 = (t * 2 + th) * 128
                ts_ = slice(th * half, (th + 1) * half)
                for j, (tc, bc) in enumerate(sl):
                    js = slice(j * half, (j + 1) * half)
                    if tc is not None:
                        xt[r0:r0 + 64, js] = xT[tc * BLK:(tc + 1) * BLK, ts_]
                    if bc is not None:
                        xt[r0 + 64:r0 + 128, js] = \
                            xT[bc * BLK:(bc + 1) * BLK, ts_]
        in_maps.append({"xt": xt, "wpk": wpk, "bias_pk": bias_pk})

    res = run_bass_kernel_spmd(nc, in_maps, core_ids=list(range(N_CORES)))
    y = np.empty((ntok_total, D_OUT), dtype=np.float32)
    for cid in range(N_CORES):
        y[cid * ntok:(cid + 1) * ntok] = \
            res.results[cid]["yt"].T.astype(np.float32)
    return y


# revision 21
# speedup vs baseline: 1.0200x; 1.0200x over previous
"""Block-sparse linear y = x @ W^T + b on 8 TRN2 NeuronCores.

Problem shape (hardcoded): x [8192, 4096] f32, weight [1024, 64, 64] f32
(64x64 blocks), bias [4096] f32, row_idx/col_idx [1024] int32 over a 64x64
block grid.

Strategy: data-parallel over tokens (1024/core); per core compute
y^T = W x^T + b with bf16 matmuls on the PE array's four 64x64 quadrants
(tile_position), which stream concurrently at the full-array rate
(measured 216ns per 4-matmul round).

x^T is SBUF-resident ONCE: col-blocks are 2-colored so every out-row has
exactly half its blocks with x in the top partition half (-> PE row-group
0) and half in the bottom (-> row-group 1); overflow columns are
duplicated at the opposite half until every row splits evenly. Out-rows
are paired (2p, 2p+1) sharing one PSUM bank per token-half th (r1 ->
array col-group 0, r2 -> col-group 1). Pair couples (A, B) run in two
slots: slot s streams A's top-blocks on row-group 0 while B's
bottom-blocks stream on row-group 1; slot s+1 swaps. Banks accumulate
across both stages; 8 banks = 2 couples in flight + 1 evicting. x lives
in [128, 4*ntok] tiles (4 top + 4 bottom col-blocks) grouped by first
use, and each stage's block order follows tile rank, so compute starts
after ~0.5MB of x has landed. Eviction is one op (psum + bias -> bf16
SBUF) alternating scalar/vector, th0 banks mid-slot; all DMA triggers
ride HWDGE queues (sync: x/weights, scalar: x/outputs) - gpsimd is
unused, avoiding its 6.4us SWDGE end-of-kernel drain. (A PE p-state
warmup spin was tried and removed: its matmuls started late on a
semaphore and ran at half clock, delaying real work ~10us.)
"""

from contextlib import ExitStack

import numpy as np
import ml_dtypes

import concourse.tile as tile
from concourse import bacc, mybir
from concourse.bass_utils import run_bass_kernel_spmd

BLK = 64
OUT_BLK = 64
IN_BLK = 64
D_IN = IN_BLK * BLK    # 4096
D_OUT = OUT_BLK * BLK  # 4096
N_CORES = 8
XPACK = 4              # col-block pairs per x SBUF tile (split by th)
BF16 = ml_dtypes.bfloat16


def _dedupe(row_idx, col_idx):
    d = {}
    for i in range(len(row_idx)):
        d[(int(row_idx[i]), int(col_idx[i]))] = i
    blocks_by_r = [[] for _ in range(OUT_BLK)]
    for (r, c), w in d.items():
        blocks_by_r[r].append((c, w))
    for lst in blocks_by_r:
        lst.sort()
    return blocks_by_r


def _balance_color(blocks_by_r, seed=0):
    """2-color the 64 col-blocks minimizing sum |#top-blocks(r) - n_r/2|."""
    Mi = np.zeros((OUT_BLK, IN_BLK), np.int64)
    for r, lst in enumerate(blocks_by_r):
        for c, _ in lst:
            Mi[r, c] = 1
    tgt = np.array([len(l) / 2.0 for l in blocks_by_r])
    best = None
    for s in range(8):
        rs = np.random.default_rng(seed + s)
        color = (rs.random(IN_BLK) < 0.5).astype(np.int8)  # 1 = top
        e = Mi[:, color == 1].sum(1).astype(float)
        c = float(np.abs(e - tgt).sum())
        T = 2.0
        for _ in range(40000):
            if c < 1e-9:
                break
            i = int(rs.integers(IN_BLK))
            ne = e + Mi[:, i] * (1 - 2 * color[i])
            ncst = float(np.abs(ne - tgt).sum())
            if ncst <= c or rs.random() < np.exp((c - ncst) / max(T, 1e-9)):
                color[i] ^= 1
                e, c = ne, ncst
            T *= 0.9997
        if best is None or c < best[0]:
            best = (c, color.copy())
        if c < 1e-9:
            break
    return best[1]


def _assign_stages(blocks_by_r, color):
    """Per-block stage (0=top/ki0, 1=bottom/ki1); duplicate overflow cols
    at the opposite half until every row splits ceil/floor(n/2)."""
    stage_of = {}
    for r, lst in enumerate(blocks_by_r):
        for c, _ in lst:
            stage_of[(r, c)] = 0 if color[c] == 1 else 1
    dup_top = set()   # cols (colored bottom) also available at a top half
    dup_bot = set()
    for _ in range(64):
        moved = False
        devs = []
        for r, lst in enumerate(blocks_by_r):
            n = len(lst)
            k0 = sum(1 for c, _ in lst if stage_of[(r, c)] == 0)
            devs.append(k0 - (n + 1) // 2 if k0 > n // 2 else k0 - n // 2
                        if k0 < n // 2 else 0)
        # free moves via existing dups
        for r, lst in enumerate(blocks_by_r):
            d = devs[r]
            while d > 0:
                c = next((c for c, _ in lst if stage_of[(r, c)] == 0
                          and c in dup_bot), None)
                if c is None:
                    break
                stage_of[(r, c)] = 1
                d -= 1
                moved = True
            while d < 0:
                c = next((c for c, _ in lst if stage_of[(r, c)] == 1
                          and c in dup_top), None)
                if c is None:
                    break
                stage_of[(r, c)] = 0
                d += 1
                moved = True
            devs[r] = d
        if all(d == 0 for d in devs):
            break
        if not moved:
            # add the dup col helping the most deficient rows
            cnt_b, cnt_t = {}, {}
            for r, lst in enumerate(blocks_by_r):
                if devs[r] > 0:
                    for c, _ in lst:
                        if stage_of[(r, c)] == 0 and c not in dup_bot:
                            cnt_b[c] = cnt_b.get(c, 0) + 1
                elif devs[r] < 0:
                    for c, _ in lst:
                        if stage_of[(r, c)] == 1 and c not in dup_top:
                            cnt_t[c] = cnt_t.get(c, 0) + 1
            if cnt_b and (not cnt_t or max(cnt_b.values())
                          >= max(cnt_t.values())):
                dup_bot.add(max(cnt_b, key=cnt_b.get))
            elif cnt_t:
                dup_top.add(max(cnt_t, key=cnt_t.get))
            else:
                break
    stage_lists = []
    for r, lst in enumerate(blocks_by_r):
        l0 = [(c, w) for c, w in lst if stage_of[(r, c)] == 0]
        l1 = [(c, w) for c, w in lst if stage_of[(r, c)] == 1]
        stage_lists.append((l0, l1))
    return stage_lists, sorted(dup_top), sorted(dup_bot)


def _couple_order(stage_lists, color):
    """Order pairs into couples; greedy to reuse col-blocks early."""
    npairs = OUT_BLK // 2
    cols_of = []
    for p in range(npairs):
        s = set()
        for r in (2 * p, 2 * p + 1):
            for ki in (0, 1):
                for c, _ in stage_lists[r][ki]:
                    s.add((c, ki))
        cols_of.append(s)
    remaining = set(range(npairs))
    order = []
    seen = set()
    cur = 0
    while remaining:
        order.append(cur)
        remaining.discard(cur)
        seen |= cols_of[cur]
        if not remaining:
            break
        cur = min(remaining, key=lambda p: len(cols_of[p] - seen))
    return order


def _layout_tiles(stage_lists, color, dup_top, dup_bot, porder):
    """Group (top-col, bottom-col) into XPACK-wide tiles by first-use slot;
    reorder each stage list by tile rank. Returns tiles, loc, stage_lists."""
    nslots = len(porder)
    first_top, first_bot = {}, {}
    for s in range(nslots):
        A, B = porder[s], porder[s ^ 1]
        for mi in (0, 1):
            for c, _ in stage_lists[2 * A + mi][0]:
                first_top.setdefault(c, (s, len(first_top)))
            for c, _ in stage_lists[2 * B + mi][1]:
                first_bot.setdefault(c, (s, len(first_bot)))
    top_cols = sorted(first_top, key=first_top.get)       # used at a top half
    bot_cols = sorted(first_bot, key=first_bot.get)
    nslots_x = max(len(top_cols), len(bot_cols))
    slots = []
    for i in range(nslots_x):
        tc = top_cols[i] if i < len(top_cols) else None
        bc = bot_cols[i] if i < len(bot_cols) else None
        slots.append((tc, bc))
    ntiles = (nslots_x + XPACK - 1) // XPACK
    tiles = [slots[t * XPACK:(t + 1) * XPACK] for t in range(ntiles)]
    loc = {}
    for t, sl in enumerate(tiles):
        for j, (tc, bc) in enumerate(sl):
            if tc is not None:
                loc[(tc, 0)] = (t, j)
            if bc is not None:
                loc[(bc, 1)] = (t, j)
    # tiles are th-split at DMA/SBUF level: tile (t, th) holds the th
    # token-half of XPACK col-pairs; loc is th-independent
    out_lists = []
    for r, (l0, l1) in enumerate(stage_lists):
        out_lists.append((sorted(l0, key=lambda cw: loc[(cw[0], 0)]),
                          sorted(l1, key=lambda cw: loc[(cw[0], 1)])))
    return tiles, loc, out_lists


def _pack_host_arrays(weight, bias, stage_lists, porder):
    nslots = len(porder)
    widths = []
    for s in range(nslots):
        A, B = porder[s], porder[s ^ 1]
        w = 1
        for mi in (0, 1):
            w = max(w, len(stage_lists[2 * A + mi][0]),
                    len(stage_lists[2 * B + mi][1]))
        widths.append(w * 2)
    offs = np.cumsum([0] + widths)
    wpk = np.zeros((128, int(offs[-1]) * BLK), dtype=BF16)
    wT = np.ascontiguousarray(
        np.transpose(np.asarray(weight), (0, 2, 1))).astype(BF16)
    for s in range(nslots):
        base = int(offs[s])
        A, B = porder[s], porder[s ^ 1]
        for ki, p in ((0, A), (1, B)):
            for mi in (0, 1):
                for b, (c, w) in enumerate(stage_lists[2 * p + mi][ki]):
                    col = (base + 2 * b + mi) * BLK
                    wpk[ki * 64:(ki + 1) * 64, col:col + BLK] = wT[w]
    bias_pk = np.zeros((128, OUT_BLK // 2), dtype=np.float32)
    for p in range(OUT_BLK // 2):
        bias_pk[0:64, p] = bias[2 * p * BLK:(2 * p + 1) * BLK]
        bias_pk[64:128, p] = bias[(2 * p + 1) * BLK:(2 * p + 2) * BLK]
    return wpk, bias_pk, offs


def _build_kernel(stage_lists, tiles, loc, porder, offs, ntok,
                  w_bufs=6, out_bufs=8):
    n_th = ntok // 512
    assert n_th == 2
    sdt = mybir.dt.bfloat16
    f32 = mybir.dt.float32
    nslots = len(porder)
    ntiles = len(tiles)

    nc = bacc.Bacc("TRN2", target_bir_lowering=False, debug=False)
    xt_d = nc.dram_tensor("xt", [ntiles * 2 * 128, XPACK * (ntok // 2)], sdt,
                          kind="ExternalInput").ap()
    w_d = nc.dram_tensor("wpk", [128, int(offs[-1]) * BLK], sdt,
                         kind="ExternalInput").ap()
    bias_d = nc.dram_tensor("bias_pk", [128, OUT_BLK // 2], f32,
                            kind="ExternalInput").ap()
    yt_d = nc.dram_tensor("yt", [D_OUT, ntok], sdt,
                          kind="ExternalOutput").ap()

    with tile.TileContext(nc) as tc:
        with ExitStack() as ctx:
            xpool = ctx.enter_context(tc.tile_pool(name="xp", bufs=1))
            wpool = ctx.enter_context(tc.tile_pool(name="wp", bufs=w_bufs))
            pspool = ctx.enter_context(
                tc.tile_pool(name="ps", bufs=8, space="PSUM"))
            opool = ctx.enter_context(tc.tile_pool(name="op", bufs=out_bufs))
            bpool = ctx.enter_context(tc.tile_pool(name="bp", bufs=1))

            bias_sb = bpool.tile([128, OUT_BLK // 2], f32, tag="bias",
                                 name="bias_sb")

            xtiles = {}
            nxdma = [0]

            def x_ap(c, ki, th):
                t, j = loc[(c, ki)]
                key = (t, th)
                if key not in xtiles:
                    tl = xpool.tile([128, XPACK * 512], sdt,
                                    tag=f"x{t}_{th}", name=f"x{t}_{th}")
                    eng = nc.sync if nxdma[0] % 2 == 0 else nc.scalar
                    nxdma[0] += 1
                    row = (t * 2 + th) * 128
                    eng.dma_start(tl[:], xt_d[row:row + 128, :])
                    xtiles[key] = tl
                tl = xtiles[key]
                o = j * 512
                return tl[ki * 64:(ki + 1) * 64, o:o + 512]

            def slot_cols(s):
                cols = []
                A, B = porder[s], porder[s ^ 1]
                for ki, p in ((0, A), (1, B)):
                    for mi in (0, 1):
                        for c, _w in stage_lists[2 * p + mi][ki]:
                            cols.append((c, ki))
                return cols

            s0_cols = slot_cols(0)
            s1_cols = slot_cols(1)

            wg_tiles = {}
            # slot 0 head: a small duplicate of wg0's first 8 block-cols so
            # the first LDWEIGHTS only waits on a 128KB transfer
            wg0a = None

            def ensure_wg(s):
                if s < nslots and s not in wg_tiles:
                    ncols = int(offs[s + 1] - offs[s]) * BLK
                    t = wpool.tile([128, ncols], sdt, tag="wg",
                                   name=f"wg{s}")
                    nc.sync.dma_start(
                        t[:], w_d[:, int(offs[s]) * BLK:
                                  int(offs[s]) * BLK + ncols])
                    wg_tiles[s] = t

            psum = {}
            mmidx = {}
            mmtot = {}
            for p in range(OUT_BLK // 2):
                for mi in (0, 1):
                    mmtot[(p, mi)] = (len(stage_lists[2 * p + mi][0])
                                      + len(stage_lists[2 * p + mi][1]))

            # Startup-critical DMA order: wg0-head, slot-0 th0 x, wg0,
            # wg1, slot-1 th0 x, bias, then th1 tiles; the slot loop
            # fetches the rest in first-use order.
            hcols = min(8, int(offs[1] - offs[0]))
            wg0a = wpool.tile([128, hcols * BLK], sdt, tag="wg",
                              name="wg0a")
            nc.sync.dma_start(
                wg0a[:], w_d[:, int(offs[0]) * BLK:
                             (int(offs[0]) + hcols) * BLK])
            ensure_wg(0)
            for c, ki in s0_cols:
                x_ap(c, ki, 0)
            ensure_wg(1)
            for c, ki in s1_cols:
                x_ap(c, ki, 0)
            nc.sync.dma_start(bias_sb[:], bias_d[:])
            for c, ki in s0_cols:
                x_ap(c, ki, 1)
            for c, ki in s1_cols:
                x_ap(c, ki, 1)

            nev = [0]

            def evict(p, th):
                pt = psum.pop((p, th))
                osb = opool.tile([128, 512], sdt, tag="o", name=f"o{p}_{th}")
                nev[0] += 1
                bcol = bias_sb[:, p:p + 1]
                n1 = mmtot[(p, 0)]
                n2 = mmtot[(p, 1)]
                if n1 > 0 and n2 > 0:
                    nc.vector.tensor_scalar_add(osb[:], pt[:], bcol)
                else:
                    for mi, nm in ((0, n1), (1, n2)):
                        oh = osb[mi * 64:(mi + 1) * 64, :]
                        bh = bias_sb[mi * 64:(mi + 1) * 64, p:p + 1]
                        if nm > 0:
                            nc.vector.tensor_scalar_add(
                                oh, pt[mi * 64:(mi + 1) * 64, :], bh)
                        else:
                            nc.vector.memset(oh, 0.0)
                            nc.vector.tensor_scalar_add(oh, oh, bh)
                nc.scalar.dma_start(
                    yt_d[2 * p * BLK:2 * p * BLK + 128,
                         th * 512:(th + 1) * 512],
                    osb[:])

            def ensure_psum(p, th):
                if (p, th) not in psum:
                    psum[(p, th)] = pspool.tile(
                        [128, 512], f32, tag="ps", name=f"ps{p}_{th}")
                    for mi in (0, 1):
                        mmidx[(p, th, mi)] = 0

            for s in range(nslots):
                ensure_wg(s)
                ensure_wg(s + 1)
                ensure_wg(s + 2)
                base = int(offs[s])
                wg = wg_tiles[s]
                A, B = porder[s], porder[s ^ 1]
                work = []
                for ki, p in ((0, A), (1, B)):
                    for mi in (0, 1):
                        work.append((ki, p, mi, stage_lists[2 * p + mi][ki]))
                    for th in range(n_th):
                        ensure_psum(p, th)
                nsteps = max((len(w[3]) for w in work), default=1)
                for th in range(n_th):
                    for b in range(nsteps):
                        for ki, p, mi, blks in work:
                            if b >= len(blks):
                                continue
                            c, _w = blks[b]
                            wsrc = wg0a if (s == 0 and 2 * b + mi < hcols) \
                                else wg
                            lhsT = wsrc[ki * 64:(ki + 1) * 64,
                                        (2 * b + mi) * BLK:
                                        (2 * b + mi + 1) * BLK]
                            i = mmidx[(p, th, mi)]
                            mmidx[(p, th, mi)] = i + 1
                            nc.tensor.matmul(
                                psum[(p, th)][mi * 64:(mi + 1) * 64, :],
                                lhsT, x_ap(c, ki, th),
                                start=(i == 0),
                                stop=(i == mmtot[(p, mi)] - 1),
                                tile_position=(ki * 64, mi * 64),
                                skip_group_check=True,
                            )
                    if s % 2 == 1:
                        # both couple pairs' th banks complete here
                        for p in (porder[s - 1], porder[s]):
                            evict(p, th)
    nc.compile()
    return nc


def kernel(x, weight, bias, row_idx, col_idx):
    x = np.asarray(x, dtype=np.float32)
    weight = np.asarray(weight, dtype=np.float32)
    bias = np.asarray(bias, dtype=np.float32)
    row_idx = np.asarray(row_idx)
    col_idx = np.asarray(col_idx)
    ntok_total = x.shape[0]
    assert ntok_total % N_CORES == 0
    ntok = ntok_total // N_CORES

    blocks_by_r = _dedupe(row_idx, col_idx)
    color = _balance_color(blocks_by_r)
    stage_lists, dup_top, dup_bot = _assign_stages(blocks_by_r, color)
    porder = _couple_order(stage_lists, color)
    tiles, loc, stage_lists = _layout_tiles(
        stage_lists, color, dup_top, dup_bot, porder)
    wpk, bias_pk, offs = _pack_host_arrays(
        weight, bias, stage_lists, porder)
    nc = _build_kernel(stage_lists, tiles, loc, porder, offs, ntok)

    in_maps = []
    for cid in range(N_CORES):
        xT = np.ascontiguousarray(
            x[cid * ntok:(cid + 1) * ntok].T).astype(BF16)  # [4096, ntok]
        half = ntok // 2
        xt = np.zeros((len(tiles) * 2 * 128, XPACK * half), dtype=BF16)
        for t, sl in enumerate(tiles):
            for th in range(2):
                r0 = (t * 2 + th) * 128
                ts_ = slice(th * half, (th + 1) * half)
                for j, (tc, bc) in enumerate(sl):
                    js = slice(j * half, (j + 1) * half)
                    if tc is not None:
                        xt[r0:r0 + 64, js] = xT[tc * BLK:(tc + 1) * BLK, ts_]
                    if bc is not None:
                        xt[r0 + 64:r0 + 128, js] = \
                            xT[bc * BLK:(bc + 1) * BLK, ts_]
        in_maps.append({"xt": xt, "wpk": wpk, "bias_pk": bias_pk})

    res = run_bass_kernel_spmd(nc, in_maps, core_ids=list(range(N_CORES)))
    y = np.empty((ntok_total, D_OUT), dtype=np.float32)
    for cid in range(N_CORES):
        y[cid * ntok:(cid + 1) * ntok] = \
            res.results[cid]["yt"].T.astype(np.float32)
    return y


# revision 22
# speedup vs baseline: 1.0352x; 1.0149x over previous
"""Block-sparse linear y = x @ W^T + b on 8 TRN2 NeuronCores.

Problem shape (hardcoded): x [8192, 4096] f32, weight [1024, 64, 64] f32
(64x64 blocks), bias [4096] f32, row_idx/col_idx [1024] int32 over a 64x64
block grid.

Strategy: data-parallel over tokens (1024/core); per core compute
y^T = W x^T + b with bf16 matmuls on the PE array's four 64x64 quadrants
(tile_position), which stream concurrently at the full-array rate
(measured 216ns per 4-matmul round).

x^T is SBUF-resident ONCE: col-blocks are 2-colored so every out-row has
exactly half its blocks with x in the top partition half (-> PE row-group
0) and half in the bottom (-> row-group 1); overflow columns are
duplicated at the opposite half until every row splits evenly. Out-rows
are paired (2p, 2p+1) sharing one PSUM bank per token-half th (r1 ->
array col-group 0, r2 -> col-group 1). Pair couples (A, B) run in two
slots: slot s streams A's top-blocks on row-group 0 while B's
bottom-blocks stream on row-group 1; slot s+1 swaps. Banks accumulate
across both stages; 8 banks = 2 couples in flight + 1 evicting. x lives
in [128, 4*ntok] tiles (4 top + 4 bottom col-blocks) grouped by first
use, and each stage's block order follows tile rank, so compute starts
after ~0.5MB of x has landed. Eviction is one op (psum + bias -> bf16
SBUF) alternating scalar/vector, th0 banks mid-slot; all DMA triggers
ride HWDGE queues (sync: x/weights, scalar: x/outputs) - gpsimd is
unused, avoiding its 6.4us SWDGE end-of-kernel drain. (A PE p-state
warmup spin was tried and removed: its matmuls started late on a
semaphore and ran at half clock, delaying real work ~10us.)
"""

from contextlib import ExitStack

import numpy as np
import ml_dtypes

import concourse.tile as tile
from concourse import bacc, mybir
from concourse.bass_utils import run_bass_kernel_spmd

BLK = 64
OUT_BLK = 64
IN_BLK = 64
D_IN = IN_BLK * BLK    # 4096
D_OUT = OUT_BLK * BLK  # 4096
N_CORES = 8
XPACK = 4              # col-block pairs per x SBUF tile (split by th)
BF16 = ml_dtypes.bfloat16


def _dedupe(row_idx, col_idx):
    d = {}
    for i in range(len(row_idx)):
        d[(int(row_idx[i]), int(col_idx[i]))] = i
    blocks_by_r = [[] for _ in range(OUT_BLK)]
    for (r, c), w in d.items():
        blocks_by_r[r].append((c, w))
    for lst in blocks_by_r:
        lst.sort()
    return blocks_by_r


def _balance_color(blocks_by_r, seed=0):
    """2-color the 64 col-blocks minimizing sum |#top-blocks(r) - n_r/2|."""
    Mi = np.zeros((OUT_BLK, IN_BLK), np.int64)
    for r, lst in enumerate(blocks_by_r):
        for c, _ in lst:
            Mi[r, c] = 1
    tgt = np.array([len(l) / 2.0 for l in blocks_by_r])
    best = None
    for s in range(8):
        rs = np.random.default_rng(seed + s)
        color = (rs.random(IN_BLK) < 0.5).astype(np.int8)  # 1 = top
        e = Mi[:, color == 1].sum(1).astype(float)
        c = float(np.abs(e - tgt).sum())
        T = 2.0
        for _ in range(40000):
            if c < 1e-9:
                break
            i = int(rs.integers(IN_BLK))
            ne = e + Mi[:, i] * (1 - 2 * color[i])
            ncst = float(np.abs(ne - tgt).sum())
            if ncst <= c or rs.random() < np.exp((c - ncst) / max(T, 1e-9)):
                color[i] ^= 1
                e, c = ne, ncst
            T *= 0.9997
        if best is None or c < best[0]:
            best = (c, color.copy())
        if c < 1e-9:
            break
    return best[1]


def _assign_stages(blocks_by_r, color):
    """Per-block stage (0=top/ki0, 1=bottom/ki1); duplicate overflow cols
    at the opposite half until every row splits ceil/floor(n/2)."""
    stage_of = {}
    for r, lst in enumerate(blocks_by_r):
        for c, _ in lst:
            stage_of[(r, c)] = 0 if color[c] == 1 else 1
    dup_top = set()   # cols (colored bottom) also available at a top half
    dup_bot = set()
    for _ in range(64):
        moved = False
        devs = []
        for r, lst in enumerate(blocks_by_r):
            n = len(lst)
            k0 = sum(1 for c, _ in lst if stage_of[(r, c)] == 0)
            devs.append(k0 - (n + 1) // 2 if k0 > n // 2 else k0 - n // 2
                        if k0 < n // 2 else 0)
        # free moves via existing dups
        for r, lst in enumerate(blocks_by_r):
            d = devs[r]
            while d > 0:
                c = next((c for c, _ in lst if stage_of[(r, c)] == 0
                          and c in dup_bot), None)
                if c is None:
                    break
                stage_of[(r, c)] = 1
                d -= 1
                moved = True
            while d < 0:
                c = next((c for c, _ in lst if stage_of[(r, c)] == 1
                          and c in dup_top), None)
                if c is None:
                    break
                stage_of[(r, c)] = 0
                d += 1
                moved = True
            devs[r] = d
        if all(d == 0 for d in devs):
            break
        if not moved:
            # add the dup col helping the most deficient rows
            cnt_b, cnt_t = {}, {}
            for r, lst in enumerate(blocks_by_r):
                if devs[r] > 0:
                    for c, _ in lst:
                        if stage_of[(r, c)] == 0 and c not in dup_bot:
                            cnt_b[c] = cnt_b.get(c, 0) + 1
                elif devs[r] < 0:
                    for c, _ in lst:
                        if stage_of[(r, c)] == 1 and c not in dup_top:
                            cnt_t[c] = cnt_t.get(c, 0) + 1
            if cnt_b and (not cnt_t or max(cnt_b.values())
                          >= max(cnt_t.values())):
                dup_bot.add(max(cnt_b, key=cnt_b.get))
            elif cnt_t:
                dup_top.add(max(cnt_t, key=cnt_t.get))
            else:
                break
    stage_lists = []
    for r, lst in enumerate(blocks_by_r):
        l0 = [(c, w) for c, w in lst if stage_of[(r, c)] == 0]
        l1 = [(c, w) for c, w in lst if stage_of[(r, c)] == 1]
        stage_lists.append((l0, l1))
    return stage_lists, sorted(dup_top), sorted(dup_bot)


def _couple_order(stage_lists, color):
    """Order pairs into couples; greedy to reuse col-blocks early."""
    npairs = OUT_BLK // 2
    cols_of = []
    for p in range(npairs):
        s = set()
        for r in (2 * p, 2 * p + 1):
            for ki in (0, 1):
                for c, _ in stage_lists[r][ki]:
                    s.add((c, ki))
        cols_of.append(s)
    remaining = set(range(npairs))
    order = []
    seen = set()
    cur = 0
    while remaining:
        order.append(cur)
        remaining.discard(cur)
        seen |= cols_of[cur]
        if not remaining:
            break
        cur = min(remaining, key=lambda p: len(cols_of[p] - seen))
    return order


def _layout_tiles(stage_lists, color, dup_top, dup_bot, porder):
    """Group (top-col, bottom-col) into XPACK-wide tiles by first-use slot;
    reorder each stage list by tile rank. Returns tiles, loc, stage_lists."""
    nslots = len(porder)
    first_top, first_bot = {}, {}
    for s in range(nslots):
        A, B = porder[s], porder[s ^ 1]
        for mi in (0, 1):
            for c, _ in stage_lists[2 * A + mi][0]:
                first_top.setdefault(c, (s, len(first_top)))
            for c, _ in stage_lists[2 * B + mi][1]:
                first_bot.setdefault(c, (s, len(first_bot)))
    top_cols = sorted(first_top, key=first_top.get)       # used at a top half
    bot_cols = sorted(first_bot, key=first_bot.get)
    nslots_x = max(len(top_cols), len(bot_cols))
    slots = []
    for i in range(nslots_x):
        tc = top_cols[i] if i < len(top_cols) else None
        bc = bot_cols[i] if i < len(bot_cols) else None
        slots.append((tc, bc))
    ntiles = (nslots_x + XPACK - 1) // XPACK
    tiles = [slots[t * XPACK:(t + 1) * XPACK] for t in range(ntiles)]
    loc = {}
    for t, sl in enumerate(tiles):
        for j, (tc, bc) in enumerate(sl):
            if tc is not None:
                loc[(tc, 0)] = (t, j)
            if bc is not None:
                loc[(bc, 1)] = (t, j)
    # tiles are th-split at DMA/SBUF level: tile (t, th) holds the th
    # token-half of XPACK col-pairs; loc is th-independent
    out_lists = []
    for r, (l0, l1) in enumerate(stage_lists):
        out_lists.append((sorted(l0, key=lambda cw: loc[(cw[0], 0)]),
                          sorted(l1, key=lambda cw: loc[(cw[0], 1)])))
    return tiles, loc, out_lists


def _pack_host_arrays(weight, bias, stage_lists, porder):
    nslots = len(porder)
    widths = []
    for s in range(nslots):
        A, B = porder[s], porder[s ^ 1]
        w = 1
        for mi in (0, 1):
            w = max(w, len(stage_lists[2 * A + mi][0]),
                    len(stage_lists[2 * B + mi][1]))
        widths.append(w * 2)
    offs = np.cumsum([0] + widths)
    wpk = np.zeros((128, int(offs[-1]) * BLK), dtype=BF16)
    wT = np.ascontiguousarray(
        np.transpose(np.asarray(weight), (0, 2, 1))).astype(BF16)
    for s in range(nslots):
        base = int(offs[s])
        A, B = porder[s], porder[s ^ 1]
        for ki, p in ((0, A), (1, B)):
            for mi in (0, 1):
                for b, (c, w) in enumerate(stage_lists[2 * p + mi][ki]):
                    col = (base + 2 * b + mi) * BLK
                    wpk[ki * 64:(ki + 1) * 64, col:col + BLK] = wT[w]
    bias_pk = np.zeros((128, OUT_BLK // 2), dtype=np.float32)
    for p in range(OUT_BLK // 2):
        bias_pk[0:64, p] = bias[2 * p * BLK:(2 * p + 1) * BLK]
        bias_pk[64:128, p] = bias[(2 * p + 1) * BLK:(2 * p + 2) * BLK]
    return wpk, bias_pk, offs


def _build_kernel(stage_lists, tiles, loc, porder, offs, ntok,
                  w_bufs=6, out_bufs=8):
    n_th = ntok // 512
    assert n_th == 2
    sdt = mybir.dt.bfloat16
    f32 = mybir.dt.float32
    nslots = len(porder)
    ntiles = len(tiles)

    nc = bacc.Bacc("TRN2", target_bir_lowering=False, debug=False)
    xt_d = nc.dram_tensor("xt", [ntiles * 2 * 128, XPACK * (ntok // 2)], sdt,
                          kind="ExternalInput").ap()
    w_d = nc.dram_tensor("wpk", [128, int(offs[-1]) * BLK], sdt,
                         kind="ExternalInput").ap()
    bias_d = nc.dram_tensor("bias_pk", [128, OUT_BLK // 2], f32,
                            kind="ExternalInput").ap()
    yt_d = nc.dram_tensor("yt", [D_OUT, ntok], sdt,
                          kind="ExternalOutput").ap()

    with tile.TileContext(nc) as tc:
        with ExitStack() as ctx:
            xpool = ctx.enter_context(tc.tile_pool(name="xp", bufs=1))
            wpool = ctx.enter_context(tc.tile_pool(name="wp", bufs=w_bufs))
            pspool = ctx.enter_context(
                tc.tile_pool(name="ps", bufs=8, space="PSUM"))
            opool = ctx.enter_context(tc.tile_pool(name="op", bufs=out_bufs))
            bpool = ctx.enter_context(tc.tile_pool(name="bp", bufs=1))

            bias_sb = bpool.tile([128, OUT_BLK // 2], f32, tag="bias",
                                 name="bias_sb")

            xtiles = {}
            nxdma = [0]

            def x_ap(c, ki, th):
                t, j = loc[(c, ki)]
                key = (t, th)
                if key not in xtiles:
                    tl = xpool.tile([128, XPACK * 512], sdt,
                                    tag=f"x{t}_{th}", name=f"x{t}_{th}")
                    eng = nc.sync if nxdma[0] % 3 == 0 else nc.scalar
                    nxdma[0] += 1
                    row = (t * 2 + th) * 128
                    eng.dma_start(tl[:], xt_d[row:row + 128, :])
                    xtiles[key] = tl
                tl = xtiles[key]
                o = j * 512
                return tl[ki * 64:(ki + 1) * 64, o:o + 512]

            def slot_cols(s):
                cols = []
                A, B = porder[s], porder[s ^ 1]
                for ki, p in ((0, A), (1, B)):
                    for mi in (0, 1):
                        for c, _w in stage_lists[2 * p + mi][ki]:
                            cols.append((c, ki))
                return cols

            s0_cols = slot_cols(0)
            s1_cols = slot_cols(1)

            wg_tiles = {}
            # slot 0 head: a small duplicate of wg0's first 8 block-cols so
            # the first LDWEIGHTS only waits on a 128KB transfer
            wg0a = None

            def ensure_wg(s):
                if s < nslots and s not in wg_tiles:
                    ncols = int(offs[s + 1] - offs[s]) * BLK
                    t = wpool.tile([128, ncols], sdt, tag="wg",
                                   name=f"wg{s}")
                    nc.sync.dma_start(
                        t[:], w_d[:, int(offs[s]) * BLK:
                                  int(offs[s]) * BLK + ncols])
                    wg_tiles[s] = t

            psum = {}
            mmidx = {}
            mmtot = {}
            for p in range(OUT_BLK // 2):
                for mi in (0, 1):
                    mmtot[(p, mi)] = (len(stage_lists[2 * p + mi][0])
                                      + len(stage_lists[2 * p + mi][1]))

            # Startup-critical DMA order: wg0-head, slot-0 th0 x, wg0,
            # wg1, slot-1 th0 x, bias, then th1 tiles; the slot loop
            # fetches the rest in first-use order.
            hcols = min(8, int(offs[1] - offs[0]))
            wg0a = wpool.tile([128, hcols * BLK], sdt, tag="wg",
                              name="wg0a")
            nc.sync.dma_start(
                wg0a[:], w_d[:, int(offs[0]) * BLK:
                             (int(offs[0]) + hcols) * BLK])
            ensure_wg(0)
            for c, ki in s0_cols:
                x_ap(c, ki, 0)
            ensure_wg(1)
            for c, ki in s1_cols:
                x_ap(c, ki, 0)
            nc.sync.dma_start(bias_sb[:], bias_d[:])
            for c, ki in s0_cols:
                x_ap(c, ki, 1)
            for c, ki in s1_cols:
                x_ap(c, ki, 1)

            nev = [0]

            def evict(p, th):
                pt = psum.pop((p, th))
                osb = opool.tile([128, 512], sdt, tag="o", name=f"o{p}_{th}")
                nev[0] += 1
                bcol = bias_sb[:, p:p + 1]
                n1 = mmtot[(p, 0)]
                n2 = mmtot[(p, 1)]
                if n1 > 0 and n2 > 0:
                    nc.vector.tensor_scalar_add(osb[:], pt[:], bcol)
                else:
                    for mi, nm in ((0, n1), (1, n2)):
                        oh = osb[mi * 64:(mi + 1) * 64, :]
                        bh = bias_sb[mi * 64:(mi + 1) * 64, p:p + 1]
                        if nm > 0:
                            nc.vector.tensor_scalar_add(
                                oh, pt[mi * 64:(mi + 1) * 64, :], bh)
                        else:
                            nc.vector.memset(oh, 0.0)
                            nc.vector.tensor_scalar_add(oh, oh, bh)
                nc.scalar.dma_start(
                    yt_d[2 * p * BLK:2 * p * BLK + 128,
                         th * 512:(th + 1) * 512],
                    osb[:])

            def ensure_psum(p, th):
                if (p, th) not in psum:
                    psum[(p, th)] = pspool.tile(
                        [128, 512], f32, tag="ps", name=f"ps{p}_{th}")
                    for mi in (0, 1):
                        mmidx[(p, th, mi)] = 0

            for s in range(nslots):
                ensure_wg(s)
                ensure_wg(s + 1)
                ensure_wg(s + 2)
                base = int(offs[s])
                wg = wg_tiles[s]
                A, B = porder[s], porder[s ^ 1]
                work = []
                for ki, p in ((0, A), (1, B)):
                    for mi in (0, 1):
                        work.append((ki, p, mi, stage_lists[2 * p + mi][ki]))
                    for th in range(n_th):
                        ensure_psum(p, th)
                nsteps = max((len(w[3]) for w in work), default=1)
                for th in range(n_th):
                    for b in range(nsteps):
                        for ki, p, mi, blks in work:
                            if b >= len(blks):
                                continue
                            c, _w = blks[b]
                            wsrc = wg0a if (s == 0 and 2 * b + mi < hcols) \
                                else wg
                            lhsT = wsrc[ki * 64:(ki + 1) * 64,
                                        (2 * b + mi) * BLK:
                                        (2 * b + mi + 1) * BLK]
                            i = mmidx[(p, th, mi)]
                            mmidx[(p, th, mi)] = i + 1
                            nc.tensor.matmul(
                                psum[(p, th)][mi * 64:(mi + 1) * 64, :],
                                lhsT, x_ap(c, ki, th),
                                start=(i == 0),
                                stop=(i == mmtot[(p, mi)] - 1),
                                tile_position=(ki * 64, mi * 64),
                                skip_group_check=True,
                            )
                    if s % 2 == 1:
                        # both couple pairs' th banks complete here
                        for p in (porder[s - 1], porder[s]):
                            evict(p, th)
    nc.compile()
    return nc


def kernel(x, weight, bias, row_idx, col_idx):
    x = np.asarray(x, dtype=np.float32)
    weight = np.asarray(weight, dtype=np.float32)
    bias = np.asarray(bias, dtype=np.float32)
    row_idx = np.asarray(row_idx)
    col_idx = np.asarray(col_idx)
    ntok_total = x.shape[0]
    assert ntok_total % N_CORES == 0
    ntok = ntok_total // N_CORES

    blocks_by_r = _dedupe(row_idx, col_idx)
    color = _balance_color(blocks_by_r)
    stage_lists, dup_top, dup_bot = _assign_stages(blocks_by_r, color)
    porder = _couple_order(stage_lists, color)
    tiles, loc, stage_lists = _layout_tiles(
        stage_lists, color, dup_top, dup_bot, porder)
    wpk, bias_pk, offs = _pack_host_arrays(
        weight, bias, stage_lists, porder)
    nc = _build_kernel(stage_lists, tiles, loc, porder, offs, ntok)

    in_maps = []
    for cid in range(N_CORES):
        xT = np.ascontiguousarray(
            x[cid * ntok:(cid + 1) * ntok].T).astype(BF16)  # [4096, ntok]
        half = ntok // 2
        xt = np.zeros((len(tiles) * 2 * 128, XPACK * half), dtype=BF16)
        for t, sl in enumerate(tiles):
            for th in range(2):
                r0 = (t * 2 + th) * 128
                ts_ = slice(th * half, (th + 1) * half)
                for j, (tc, bc) in enumerate(sl):
                    js = slice(j * half, (j + 1) * half)
                    if tc is not None:
                        xt[r0:r0 + 64, js] = xT[tc * BLK:(tc + 1) * BLK, ts_]
                    if bc is not None:
                        xt[r0 + 64:r0 + 128, js] = \
                            xT[bc * BLK:(bc + 1) * BLK, ts_]
        in_maps.append({"xt": xt, "wpk": wpk, "bias_pk": bias_pk})

    res = run_bass_kernel_spmd(nc, in_maps, core_ids=list(range(N_CORES)))
    y = np.empty((ntok_total, D_OUT), dtype=np.float32)
    for cid in range(N_CORES):
        y[cid * ntok:(cid + 1) * ntok] = \
            res.results[cid]["yt"].T.astype(np.float32)
    return y
